# revision 40
# baseline (speedup 1.0000x reference)
"""Multi-head self-attention on 8 Trainium2 NeuronCores.

Sharding: tensor-parallel over heads (2 heads per core, both batch elements
on every core). Each core computes qkv projection / attention / its slice of
the output projection (rows of W_out for its heads), producing a partial
[B, N, D] output (bf16); the host sums the 8 partials in f32 and adds b_out.

Per-core dataflow, tuned for ACT-bound pacing with a gap-free PE stream:

  - QKV^T = Wsel^T @ x^T -> Q^T, K^T, V^T, each [128 = 2 heads x 64 dp, N]
  - attention runs over 128-key blocks x 512-q windows. Scores for the two
    heads use disjoint PE row-tiles (K=64 dp rows at partition 0/64) and
    land side by side in one [128 keys, 1024 = h0 q | h1 q] psum tile, so
    the head pair executes concurrently and a single exp (ScalarE,
    scale=1/sqrt(dp)) covers both heads: 128 exps of [128,1024] total, which
    is the ~143us pacing floor of the kernel.
  - V^T is PE-transposed per 128-key block into VA [128 keys, 64+1+64+1]
    with per-head ones columns; PV matmuls (K=128, M=65) produce the softmax
    row-sums for free (psum partition 64), accumulating in two single-bank
    [65, 512] psum tiles with a 3-block lag behind the exp.
  - normalize via DVE stage + DMA shift + gpsimd partition_broadcast +
    fast-approx reciprocal + multiply, DMA-shift into the combined O^T tile
  - y_partial = O^T-block^T @ W_out_slice, evacuated bf16 and DMA'd out.

Scheduling: 3 rotating scores-psum slots (6 banks) + 2 PV accumulators
(2 banks) fill all 8 PSUM banks. All other PE work (the second batch's QKV
projection in 4-matmul quarter-atoms merged via DVE adds, V-transposes,
output-projection blocks) time-shares the scores slots as ~1us filler atoms
popped one per window: per-window PE core work (~700ns) + one atom stays at
or under the 1114ns exp, so the activation engine never starves. The second
batch's x^T DMA is deferred into the first windows to keep the head
DMA-gated only by batch 0.
"""

import numpy as np
import ml_dtypes

B, N, D, H, DP = 2, 2048, 1024, 16, 64
SCALE = float(DP) ** 0.5
NCORES = 8
HC = H // NCORES            # heads per core = 2
E = HC * DP                 # per-core head-dim total = 128
KB = N // 128               # 16 key blocks
QW = 512                    # q columns per attention pass
NQW = N // QW               # 4
DC = D // 128               # 8 contraction chunks for the qkv projection
RING = 8                    # PT ring depth (key blocks)
PVLAG = 5                   # steady-state PV lag behind scores (windows)
VAW = 2 * (DP + 1)          # VA width: [V0 | 1 | V1 | 1]

BF16 = ml_dtypes.bfloat16

_CACHE = {}


def _build_bass(with_bias=False):
    import concourse.bass as bass
    import concourse.mybir as mybir
    import concourse.tile as tile
    from concourse import bacc
    from concourse.masks import make_identity

    MM_DT = mybir.dt.bfloat16    # matmul input dtype
    P_DT = mybir.dt.bfloat16     # exp(S^T) storage dtype
    F32 = mybir.dt.float32

    # nonzero b_qkv is handled by an extra contraction chunk whose x^T rows
    # are [ones, 0...] and whose weight rows carry the bias (bias as matmul)
    DCX = DC + (1 if with_bias else 0)
    DCH = (DCX + 1) // 2    # first-half dc count for quarter-atom qkv fills
    nc = bacc.Bacc(None, target_bir_lowering=False)
    xt = nc.dram_tensor("xt", [B, DCX * 128, N], MM_DT, kind="ExternalInput")[:]
    wsel = nc.dram_tensor("wsel", [128, DCX * 3 * E], MM_DT, kind="ExternalInput")[:]
    wout = nc.dram_tensor("wout", [E, D], MM_DT, kind="ExternalInput")[:]
    y = nc.dram_tensor("y", [B, N, D], MM_DT, kind="ExternalOutput")[:]
    debug = _CACHE.get("debug", False)
    if debug:
        dqkv = nc.dram_tensor("dqkv", [B, 3, 128, N], MM_DT, kind="ExternalOutput")[:]
        dva = nc.dram_tensor("dva", [B, 128, KB, VAW], MM_DT, kind="ExternalOutput")[:]
        dot = nc.dram_tensor("dot", [B, 128, N], MM_DT, kind="ExternalOutput")[:]

    with tile.TileContext(nc) as tc:
        with (
            tc.tile_pool(name="consts", bufs=1) as consts,
            tc.tile_pool(name="xtp", bufs=2) as xtp,
            tc.tile_pool(name="qkvp", bufs=2) as qkvp,
            tc.tile_pool(name="vap", bufs=2) as vap,
            tc.tile_pool(name="ptp", bufs=2) as ptp,
            tc.tile_pool(name="otp", bufs=2) as otp,
            tc.tile_pool(name="evacp", bufs=2) as evacp,
            tc.tile_pool(name="qtmpp", bufs=2) as qtmpp,
            tc.tile_pool(name="normp", bufs=2) as normp,
            # 3 rotating scores/filler slots (2 banks each) + 2 PV
            # accumulators (1 bank each) = all 8 PSUM banks
            tc.tile_pool(name="ps_sc", bufs=3, space="PSUM") as ps_sc,
            tc.tile_pool(name="ps_pv", bufs=1, space="PSUM") as ps_pv,
        ):
            WS = consts.tile([128, DCX, 3 * E], MM_DT)
            WOUT = consts.tile([128, D], MM_DT)
            IDENT = consts.tile([128, 128], MM_DT)
            make_identity(nc, IDENT)
            WARM = consts.tile([1, 1], F32)
            nc.vector.memset(WARM, 0.0)
            nc.scalar.activation(
                out=WARM, in_=WARM, func=mybir.ActivationFunctionType.Exp
            )
            # ramp the PE clock out of its low p-state while the input DMAs
            # stream in: ~40 dependency-free matmuls on the identity keep the
            # array continuously busy past the ~3us ramp threshold
            wps = ps_sc.tile([128, 1024], F32, tag="sc", name="ps_warm")
            for i in range(40):
                nc.tensor.matmul(
                    wps[:, 0:128], lhsT=IDENT, rhs=IDENT,
                    start=(i == 0), stop=(i == 39),
                )

            # ---- persistent SBUF tiles (batch 0 x^T DMA'd now; batch 1
            # deferred into the first attention windows). DMA issue order is
            # chosen so the prep matmuls start as early as possible: first
            # x^T slab, weights, remaining slabs; WOUT (first read by the
            # projection ~20 windows in) goes last.
            XTs, QKVTs, VAs, OTs = [], [], [], []
            for b in range(B):
                XT = xtp.tile([128, DCX, N], MM_DT, tag="xt", name=f"xt{b}")
                if b == 0:
                    xtb = xt[b].rearrange("(dc p) n -> p dc n", p=128)
                    nc.sync.dma_start(out=XT[:, 0, :], in_=xtb[:, 0, :])
                    nc.sync.dma_start(out=WS, in_=wsel)
                    for dc in range(1, DCX):
                        nc.sync.dma_start(out=XT[:, dc, :], in_=xtb[:, dc, :])
                    nc.sync.dma_start(out=WOUT, in_=wout)
                XTs.append(XT)
                QKVTs.append(
                    [
                        qkvp.tile([128, N], MM_DT, tag=f"qkv{eb}", name=f"qkv{eb}")
                        for eb in range(3)
                    ]
                )
                # V key-blocks: [V_h0(64) | ones | V_h1(64) | ones]
                VA = vap.tile([128, KB, VAW], MM_DT, tag="va", name="va")
                nc.gpsimd.memset(VA[:, :, DP : DP + 1], 1.0)
                nc.gpsimd.memset(VA[:, :, VAW // 2 + DP : VAW // 2 + DP + 1], 1.0)
                VAs.append(VA)
                OTs.append(otp.tile([128, N], MM_DT, tag="ot", name="ot"))

            def dma_xt_slab(b2, dc):
                xtb = xt[b2].rearrange("(dc p) n -> p dc n", p=128)
                nc.sync.dma_start(out=XTs[b2][:, dc, :], in_=xtb[:, dc, :])

            # ---- PE work emitters -------------------------------------
            # All qkv-region emissions are tracked in `qkv_done` and can be
            # force-emitted on demand (ensure_qkv) by whatever reads them, so
            # emission order (= dependency order for the tile framework) is
            # structurally correct no matter how the filler queue drifts.
            qkv_done = set()
            _qtmp = {}

            def emit_qkv_use(b2, eb, nk):
                """full qkv projection block: [128, 1024] out, 16 matmuls"""
                qkv_done.add((b2, eb, nk, 0))
                qkv_done.add((b2, eb, nk, 1))
                ps = ps_sc.tile([128, 1024], F32, tag="sc", name="ps_qkv")
                for dc in range(DCX):
                    for hf in range(2):
                        nc.tensor.matmul(
                            ps[:, hf * 512 : (hf + 1) * 512],
                            lhsT=WS[:, dc, eb * 128 : (eb + 1) * 128],
                            rhs=XTs[b2][
                                :, dc,
                                nk * 1024 + hf * 512 : nk * 1024 + (hf + 1) * 512,
                            ],
                            start=(dc == 0),
                            stop=(dc == DCX - 1),
                        )
                nc.vector.tensor_copy(
                    out=QKVTs[b2][eb][:, nk * 1024 : (nk + 1) * 1024], in_=ps
                )

            def emit_qkv_half(b2, eb, nk, hf):
                """qkv projection for a 512-col slice, full contraction"""
                qkv_done.add((b2, eb, nk, hf))
                ps = ps_sc.tile([128, 1024], F32, tag="sc", name="ps_qh")
                for dc in range(DCX):
                    nc.tensor.matmul(
                        ps[:, 0:512],
                        lhsT=WS[:, dc, eb * 128 : (eb + 1) * 128],
                        rhs=XTs[b2][
                            :, dc,
                            nk * 1024 + hf * 512 : nk * 1024 + (hf + 1) * 512,
                        ],
                        start=(dc == 0),
                        stop=(dc == DCX - 1),
                    )
                nc.vector.tensor_copy(
                    out=QKVTs[b2][eb][
                        :, nk * 1024 + hf * 512 : nk * 1024 + (hf + 1) * 512
                    ],
                    in_=ps[:, 0:512],
                )

            def emit_qkv_quarter(b2, eb, nk, hf, half):
                """qkv quarter-atom: half the contraction for a 512-col slice.
                half 0 parks partials in SBUF f32; half 1 adds and stores.
                No-op if the region was already force-emitted."""
                if (b2, eb, nk, hf) in qkv_done:
                    return
                ps = ps_sc.tile([128, 1024], F32, tag="sc", name="ps_qq")
                dcs = list(range(DCH)) if half == 0 else list(range(DCH, DCX))
                for dc in dcs:
                    nc.tensor.matmul(
                        ps[:, 0:512],
                        lhsT=WS[:, dc, eb * 128 : (eb + 1) * 128],
                        rhs=XTs[b2][
                            :, dc,
                            nk * 1024 + hf * 512 : nk * 1024 + (hf + 1) * 512,
                        ],
                        start=(dc == dcs[0]),
                        stop=(dc == dcs[-1]),
                    )
                if half == 0:
                    qt = qtmpp.tile([128, 512], F32, tag="qtmp", name="qtmp")
                    nc.vector.tensor_copy(out=qt, in_=ps[:, 0:512])
                    _qtmp[b2, eb, nk, hf] = qt
                else:
                    qkv_done.add((b2, eb, nk, hf))
                    qt = _qtmp.pop((b2, eb, nk, hf))
                    nc.vector.tensor_tensor(
                        out=QKVTs[b2][eb][
                            :, nk * 1024 + hf * 512 : nk * 1024 + (hf + 1) * 512
                        ],
                        in0=ps[:, 0:512],
                        in1=qt,
                        op=mybir.AluOpType.add,
                    )

            def ensure_qkv(b2, eb, nk, hf):
                """force a qkv region into existence before a reader"""
                if (b2, eb, nk, hf) in qkv_done:
                    return
                if (b2, eb, nk, hf) in _qtmp:
                    emit_qkv_quarter(b2, eb, nk, hf, 1)
                else:
                    emit_qkv_half(b2, eb, nk, hf)

            # vtrans work is tracked in a to-do set so a PV that needs a
            # block not yet emitted can force it (emission order IS
            # dependency order for the tile framework)
            vtrans_todo = {b2: set(range(KB)) for b2 in range(B)}

            def emit_vtrans(b2, kc):
                """transpose V^T key-block kc into VA (both heads at once)"""
                if kc not in vtrans_todo[b2]:
                    return
                vtrans_todo[b2].discard(kc)
                ensure_qkv(b2, 2, kc // 8, (kc // 4) % 2)
                pst = ps_sc.tile([128, 1024], MM_DT, tag="sc", name="ps_vt")
                nc.tensor.transpose(
                    pst[:, 0:128],
                    QKVTs[b2][2][:, kc * 128 : (kc + 1) * 128],
                    IDENT,
                )
                nc.vector.tensor_copy(
                    out=VAs[b2][:, kc, 0:DP], in_=pst[:, 0:DP]
                )
                nc.vector.tensor_copy(
                    out=VAs[b2][:, kc, VAW // 2 : VAW // 2 + DP],
                    in_=pst[:, DP : 2 * DP],
                )

            def emit_proj_block(b2, nb):
                """output projection for 128 tokens"""
                py = ps_sc.tile([128, 1024], F32, tag="sc", name="py")
                for k in range(2):
                    nc.tensor.matmul(
                        py[:, k * 512 : (k + 1) * 512],
                        lhsT=OTs[b2][:, nb * 128 : (nb + 1) * 128],
                        rhs=WOUT[:, k * 512 : (k + 1) * 512],
                        start=True,
                        stop=True,
                    )
                ysb = evacp.tile([128, D], MM_DT, tag="ysb", name="ysb")
                nc.vector.tensor_copy(out=ysb, in_=py)
                nc.sync.dma_start(out=y[b2, nb * 128 : (nb + 1) * 128, :], in_=ysb)

            # ---- filler queues: primary (deadline prep work), lazy (proj).
            # Every 4th pop prefers lazy so projection blocks spread through
            # the windows instead of bunching at the batch boundary.
            primary, lazy = [], []
            _popn = [0]

            def pop_filler():
                _popn[0] += 1
                if lazy and (_popn[0] % 4 == 0 or not primary):
                    lazy.pop(0)()
                elif primary:
                    primary.pop(0)()

            # ---- prep phase: the bare minimum the first windows need —
            # K nk0 (scores kc 0-7), Q and V first 512 columns — interleaved
            # per contraction chunk so the matmuls chase the x^T slab DMAs
            # instead of serializing behind the last one. Everything else is
            # filler atoms, deadline-ordered; readers force-emit anything
            # still missing, so ordering is structurally safe.
            qkv_done.update({(0, 1, 0, 0), (0, 1, 0, 1), (0, 0, 0, 0),
                             (0, 2, 0, 0)})
            psK = ps_sc.tile([128, 1024], F32, tag="sc", name="psK")
            psQ = ps_sc.tile([128, 1024], F32, tag="sc", name="psQ")
            psV = ps_sc.tile([128, 1024], F32, tag="sc", name="psV")
            for dc in range(DCX):
                for ps2, eb, hf in (
                    (psK, 1, 0), (psK, 1, 1), (psQ, 0, 0), (psV, 2, 0),
                ):
                    nc.tensor.matmul(
                        ps2[:, hf * 512 : (hf + 1) * 512],
                        lhsT=WS[:, dc, eb * 128 : (eb + 1) * 128],
                        rhs=XTs[0][:, dc, hf * 512 : (hf + 1) * 512],
                        start=(dc == 0),
                        stop=(dc == DCX - 1),
                    )
            nc.vector.tensor_copy(out=QKVTs[0][1][:, 0:1024], in_=psK)
            nc.vector.tensor_copy(out=QKVTs[0][0][:, 0:512], in_=psQ[:, 0:512])
            nc.vector.tensor_copy(out=QKVTs[0][2][:, 0:512], in_=psV[:, 0:512])

            # b0 leftovers, deadline-ordered (~one pop per window)
            for kc in range(4):
                primary.append(lambda kc=kc: emit_vtrans(0, kc))
            for hf in range(2):      # K nk1 (scores kc=8 at window 8)
                for half in range(2):
                    primary.append(
                        lambda hf=hf, half=half: emit_qkv_quarter(0, 1, 1, hf, half)
                    )
            for half in range(2):     # V cols 512:1024
                primary.append(
                    lambda half=half: emit_qkv_quarter(0, 2, 0, 1, half)
                )
            for half in range(2):     # Q cols 512:1024
                primary.append(
                    lambda half=half: emit_qkv_quarter(0, 0, 0, 1, half)
                )
            for hf in range(2):      # V nk1 (vtrans 8+ at window ~13)
                for half in range(2):
                    primary.append(
                        lambda hf=hf, half=half: emit_qkv_quarter(0, 2, 1, hf, half)
                    )
            for kc in range(4, 16):
                primary.append(lambda kc=kc: emit_vtrans(0, kc))
            for hf in range(2):      # Q nk1 (q window 2 at window 32)
                for half in range(2):
                    primary.append(
                        lambda hf=hf, half=half: emit_qkv_quarter(0, 0, 1, hf, half)
                    )
            # b1 prep (deadline: before b1 attention)
            for eb, nk in ((1, 0), (1, 1), (0, 0), (2, 0), (2, 1)):
                for hf in range(2):
                    for half in range(2):
                        primary.append(
                            lambda eb=eb, nk=nk, hf=hf, half=half: emit_qkv_quarter(
                                1, eb, nk, hf, half
                            )
                        )
            for kc in range(16):
                primary.append(lambda kc=kc: emit_vtrans(1, kc))
            for hf in range(2):
                for half in range(2):
                    primary.append(
                        lambda hf=hf, half=half: emit_qkv_quarter(1, 0, 1, hf, half)
                    )

            # ---- attention ---------------------------------------------
            # PV matmuls and the normalize chain trail the scores/exp stream
            # through a global pending queue that carries across q-window and
            # batch boundaries, so the exp stream never pauses. Windows where
            # the queue runs deep (just after a boundary) emit two items.
            pending = []

            def emit_norm(pvs, OT, q0, last=False):
                for h in range(HC):
                    pv = pvs[h]
                    # custom-DVE reciprocal can't read PSUM and only works
                    # on partition-base-0 tiles: stage the raw denominator
                    # row to SBUF, shift to partition 0, broadcast, THEN
                    # fast-reciprocal the [64, QW] tile. The unnormalized
                    # O^T is copied out up front so the single-buffered pv
                    # psum frees for the next q-window immediately.
                    stg = normp.tile([DP + 1, QW], F32, tag="stg", name="stg")
                    nc.vector.tensor_copy(
                        out=stg[DP : DP + 1, :], in_=pv[DP : DP + 1, :]
                    )
                    if last:
                        ocp = pv[0:DP, :]
                    else:
                        ocp = normp.tile([DP, QW], F32, tag="ocp", name="ocp")
                        nc.vector.tensor_copy(out=ocp, in_=pv[0:DP, :])
                    rt = normp.tile([1, QW], F32, tag="rt", name="rt")
                    nc.sync.dma_start(out=rt, in_=stg[DP : DP + 1, :])
                    bc = normp.tile([DP, QW], F32, tag="bc", name="bc")
                    nc.gpsimd.partition_broadcast(bc, rt)
                    rc = normp.tile([DP, QW], F32, tag="rc", name="rc")
                    nc.vector.reciprocal_approx_fast(out=rc, in_=bc)
                    ots = normp.tile([DP, QW], MM_DT, tag="ots", name="ots")
                    nc.vector.tensor_mul(out=ots, in0=ocp, in1=rc)
                    nc.sync.dma_start(
                        out=OT[h * DP : (h + 1) * DP, q0 : q0 + QW], in_=ots
                    )
                # the q-window's projection blocks may only be enqueued once
                # the O^T columns they read have been emitted (program order
                # = dependency order), i.e. right here
                b2 = OTs.index(OT)
                for nb in range(q0 // 128, (q0 + QW) // 128):
                    lazy.append(lambda b2=b2, nb=nb: emit_proj_block(b2, nb))

            for b in range(B):
                QT, KT, VT = QKVTs[b]
                VA = VAs[b]
                OT = OTs[b]
                for qw in range(NQW):
                    q0 = qw * QW
                    PT = ptp.tile([128, RING, 1024], P_DT, tag="pt", name="pt")
                    pvs = [
                        ps_pv.tile([DP + 1, QW], F32, tag=f"pv{h}", name=f"pv{h}")
                        for h in range(HC)
                    ]

                    def emit_pv(kc, b=b, pvs=pvs, PT=PT, VA=VA):
                        emit_vtrans(b, kc)  # no-op unless still pending
                        for h in range(HC):
                            nc.tensor.matmul(
                                pvs[h][0 : DP + 1, :],
                                lhsT=VA[
                                    :, kc,
                                    h * (VAW // 2) : h * (VAW // 2) + DP + 1,
                                ],
                                rhs=PT[:, kc % RING, h * 512 : (h + 1) * 512],
                                start=(kc == 0),
                                stop=(kc == KB - 1),
                            )

                    for kc in range(KB):
                        ensure_qkv(b, 0, qw // 2, qw % 2)        # Q columns
                        ensure_qkv(b, 1, kc // 8, (kc // 4) % 2)  # K columns
                        S = ps_sc.tile([128, 1024], F32, tag="sc", name="s")
                        for h in range(HC):
                            nc.tensor.matmul(
                                S[:, h * 512 : (h + 1) * 512],
                                lhsT=KT[
                                    h * 64 : (h + 1) * 64,
                                    kc * 128 : (kc + 1) * 128,
                                ],
                                rhs=QT[h * 64 : (h + 1) * 64, q0 : q0 + QW],
                                start=True,
                                stop=True,
                            )
                        nc.scalar.activation(
                            out=PT[:, kc % RING, :],
                            in_=S,
                            func=mybir.ActivationFunctionType.Exp,
                            scale=1.0 / SCALE,
                        )
                        # defer b1's x^T input DMA under b0's first windows
                        if b == 0 and qw == 0 and 2 <= kc < 2 + DCX:
                            dma_xt_slab(1, kc - 2)
                        pending.append(lambda kc=kc, f=emit_pv: f(kc))
                        # steady-state: one pending item per window at lag
                        # PVLAG+1 (ring depth 8 leaves 2 windows of slack);
                        # the +1 keeps the first PV of a fresh q-window far
                        # enough behind the previous norm that the
                        # single-buffered pv psum has been read out. Pending
                        # runs BEFORE the filler pop so the norm's pv-freeing
                        # DVE copies aren't stuck behind filler evacuations,
                        # and the filler pop is skipped entirely while a
                        # boundary backlog is burning down.
                        thr = 2 if (b == B - 1 and qw == NQW - 1) else PVLAG + 1
                        deep = len(pending) > thr + 1
                        while len(pending) > thr:
                            pending.pop(0)()
                        if not deep:
                            pop_filler()
                    lastq = b == B - 1 and qw == NQW - 1
                    pending.append(
                        lambda pvs=pvs, OT=OT, q0=q0, lastq=lastq: emit_norm(
                            pvs, OT, q0, last=lastq
                        )
                    )

            # drain pending PV/norm work first (the final norm gates the
            # last projection blocks), then remaining fillers; a few
            # dependency-free matmuls keep the PE clock up through the final
            # norm-chain latency
            while pending:
                pending.pop(0)()
            tps = ps_sc.tile([128, 1024], F32, tag="sc", name="ps_tw")
            for i in range(12):
                nc.tensor.matmul(
                    tps[:, 0:128], lhsT=IDENT, rhs=IDENT,
                    start=(i == 0), stop=(i == 11),
                )
            while primary or lazy:
                pop_filler()
            if debug:
                for b2 in range(B):
                    for eb in range(3):
                        nc.sync.dma_start(out=dqkv[b2, eb], in_=QKVTs[b2][eb])
                    nc.sync.dma_start(out=dva[b2], in_=VAs[b2])
                    nc.sync.dma_start(out=dot[b2], in_=OTs[b2])
    nc.finalize()
    return nc


def _get_bass(with_bias=False):
    key = f"nc{int(with_bias)}"
    if key not in _CACHE:
        _CACHE[key] = _build_bass(with_bias)
    return _CACHE[key]


def _make_in_maps(x, W_qkv, b_qkv, W_out):
    """Shard the full inputs into the 8 per-core input dicts."""
    x = np.asarray(x, dtype=np.float32)
    W_qkv = np.asarray(W_qkv, dtype=np.float32)
    b_qkv = np.asarray(b_qkv, dtype=np.float32)
    W_out = np.asarray(W_out, dtype=np.float32)

    with_bias = bool(np.any(b_qkv))
    # x^T per batch, shared by all cores (+ optional bias chunk rows)
    xtt = x.transpose(0, 2, 1)
    if with_bias:
        aug = np.zeros((B, 128, N), dtype=np.float32)
        aug[:, 0, :] = 1.0
        xtt = np.concatenate([xtt, aug], axis=1)
    xt = np.ascontiguousarray(xtt).astype(BF16)

    in_maps = []
    for c in range(NCORES):
        heads = [HC * c + i for i in range(HC)]
        # W_qkv columns: head h occupies cols [h*3*DP, (h+1)*3*DP) as [q|k|v]
        qcols = [W_qkv[:, h * 3 * DP : h * 3 * DP + DP] for h in heads]
        kcols = [W_qkv[:, h * 3 * DP + DP : h * 3 * DP + 2 * DP] for h in heads]
        vcols = [W_qkv[:, h * 3 * DP + 2 * DP : h * 3 * DP + 3 * DP] for h in heads]
        wsel = np.concatenate(qcols + kcols + vcols, axis=1)  # [D, 3*E]
        if with_bias:
            bq = [b_qkv[h * 3 * DP : h * 3 * DP + DP] for h in heads]
            bk = [b_qkv[h * 3 * DP + DP : h * 3 * DP + 2 * DP] for h in heads]
            bv = [b_qkv[h * 3 * DP + 2 * DP : h * 3 * DP + 3 * DP] for h in heads]
            brow = np.concatenate(bq + bk + bv)  # [3*E]
            baug = np.zeros((128, 3 * E), dtype=np.float32)
            baug[0, :] = brow
            wsel = np.concatenate([wsel, baug], axis=0)
        woutc = np.concatenate(
            [W_out[h * DP : (h + 1) * DP, :] for h in heads], axis=0
        )  # [E, D]
        dcx = wsel.shape[0] // 128
        wsel_r = wsel.reshape(dcx, 128, wsel.shape[1]).transpose(1, 0, 2)
        in_maps.append(
            {
                "xt": xt,
                "wsel": np.ascontiguousarray(wsel_r.reshape(128, -1)).astype(BF16),
                "wout": np.ascontiguousarray(woutc).astype(BF16),
            }
        )
    return in_maps, with_bias


def _run(in_maps, with_bias=False, trace=False):
    from concourse import bass_utils

    nc = _get_bass(with_bias)
    return bass_utils.run_bass_kernel_spmd(
        nc, in_maps, core_ids=list(range(NCORES)), trace=trace
    )


def kernel(x, W_qkv, b_qkv, W_out, b_out, _trace=False):
    in_maps, with_bias = _make_in_maps(x, W_qkv, b_qkv, W_out)
    res = _run(in_maps, with_bias=with_bias, trace=_trace)
    y = np.zeros((B, N, D), dtype=np.float32)
    for r in res.results:
        y += np.asarray(r["y"], dtype=np.float32)
    y += np.asarray(b_out, dtype=np.float32)
    _CACHE["last_result"] = res
    return y


# revision 41
# speedup vs baseline: 1.0216x; 1.0216x over previous
"""Multi-head self-attention on 8 Trainium2 NeuronCores.

Sharding: tensor-parallel over heads (2 heads per core, both batch elements
on every core). Each core computes qkv projection / attention / its slice of
the output projection (rows of W_out for its heads), producing a partial
[B, N, D] output (bf16); the host sums the 8 partials in f32 and adds b_out.

Per-core dataflow, tuned for ACT-bound pacing with a gap-free PE stream:

  - QKV^T = Wsel^T @ x^T -> Q^T, K^T, V^T, each [128 = 2 heads x 64 dp, N]
  - attention runs over 128-key blocks x 512-q windows. Scores for the two
    heads use disjoint PE row-tiles (K=64 dp rows at partition 0/64) and
    land side by side in one [128 keys, 1024 = h0 q | h1 q] psum tile, so
    the head pair executes concurrently and a single exp (ScalarE,
    scale=1/sqrt(dp)) covers both heads: 128 exps of [128,1024] total, which
    is the ~143us pacing floor of the kernel.
  - V^T is PE-transposed per 128-key block into VA [128 keys, 64+1+64+1]
    with per-head ones columns; PV matmuls (K=128, M=65) produce the softmax
    row-sums for free (psum partition 64), accumulating in two single-bank
    [65, 512] psum tiles with a 3-block lag behind the exp.
  - normalize via DVE stage + DMA shift + gpsimd partition_broadcast +
    fast-approx reciprocal + multiply, DMA-shift into the combined O^T tile
  - y_partial = O^T-block^T @ W_out_slice, evacuated bf16 and DMA'd out.

Scheduling: 3 rotating scores-psum slots (6 banks) + 2 PV accumulators
(2 banks) fill all 8 PSUM banks. All other PE work (the second batch's QKV
projection in 4-matmul quarter-atoms merged via DVE adds, V-transposes,
output-projection blocks) time-shares the scores slots as ~1us filler atoms
popped one per window: per-window PE core work (~700ns) + one atom stays at
or under the 1114ns exp, so the activation engine never starves. The second
batch's x^T DMA is deferred into the first windows to keep the head
DMA-gated only by batch 0.
"""

import numpy as np
import ml_dtypes

B, N, D, H, DP = 2, 2048, 1024, 16, 64
SCALE = float(DP) ** 0.5
NCORES = 8
HC = H // NCORES            # heads per core = 2
E = HC * DP                 # per-core head-dim total = 128
KB = N // 128               # 16 key blocks
QW = 512                    # q columns per attention pass
NQW = N // QW               # 4
DC = D // 128               # 8 contraction chunks for the qkv projection
RING = 8                    # PT ring depth (key blocks)
PVLAG = 5                   # steady-state PV lag behind scores (windows)
VAW = 2 * (DP + 1)          # VA width: [V0 | 1 | V1 | 1]

BF16 = ml_dtypes.bfloat16

_CACHE = {}


def _build_bass(with_bias=False):
    import concourse.bass as bass
    import concourse.mybir as mybir
    import concourse.tile as tile
    from concourse import bacc
    from concourse.masks import make_identity

    MM_DT = mybir.dt.bfloat16    # matmul input dtype
    P_DT = mybir.dt.bfloat16     # exp(S^T) storage dtype
    F32 = mybir.dt.float32

    # nonzero b_qkv is handled by an extra contraction chunk whose x^T rows
    # are [ones, 0...] and whose weight rows carry the bias (bias as matmul)
    DCX = DC + (1 if with_bias else 0)
    DCH = (DCX + 1) // 2    # first-half dc count for quarter-atom qkv fills
    nc = bacc.Bacc(None, target_bir_lowering=False)
    xt = nc.dram_tensor("xt", [B, DCX * 128, N], MM_DT, kind="ExternalInput")[:]
    wsel = nc.dram_tensor("wsel", [128, DCX * 3 * E], MM_DT, kind="ExternalInput")[:]
    wout = nc.dram_tensor("wout", [E, D], MM_DT, kind="ExternalInput")[:]
    y = nc.dram_tensor("y", [B, N, D], MM_DT, kind="ExternalOutput")[:]
    debug = _CACHE.get("debug", False)
    if debug:
        dqkv = nc.dram_tensor("dqkv", [B, 3, 128, N], MM_DT, kind="ExternalOutput")[:]
        dva = nc.dram_tensor("dva", [B, 128, KB, VAW], MM_DT, kind="ExternalOutput")[:]
        dot = nc.dram_tensor("dot", [B, 128, N], MM_DT, kind="ExternalOutput")[:]

    with tile.TileContext(nc) as tc:
        with (
            tc.tile_pool(name="consts", bufs=1) as consts,
            tc.tile_pool(name="xtp", bufs=2) as xtp,
            tc.tile_pool(name="qkvp", bufs=2) as qkvp,
            tc.tile_pool(name="vap", bufs=2) as vap,
            tc.tile_pool(name="ptp", bufs=2) as ptp,
            tc.tile_pool(name="otp", bufs=2) as otp,
            tc.tile_pool(name="evacp", bufs=2) as evacp,
            tc.tile_pool(name="qtmpp", bufs=2) as qtmpp,
            tc.tile_pool(name="normp", bufs=2) as normp,
            # 3 rotating scores/filler slots (2 banks each) + 2 PV
            # accumulators (1 bank each) = all 8 PSUM banks
            tc.tile_pool(name="ps_sc", bufs=3, space="PSUM") as ps_sc,
            tc.tile_pool(name="ps_pv", bufs=1, space="PSUM") as ps_pv,
        ):
            WS = consts.tile([128, DCX, 3 * E], MM_DT)
            WOUT = consts.tile([128, D], MM_DT)
            IDENT = consts.tile([128, 128], MM_DT)
            make_identity(nc, IDENT)
            WARM = consts.tile([1, 1], F32)
            nc.vector.memset(WARM, 0.0)
            nc.scalar.activation(
                out=WARM, in_=WARM, func=mybir.ActivationFunctionType.Exp
            )
            # ramp the PE clock out of its low p-state while the input DMAs
            # stream in: ~40 dependency-free matmuls on the identity keep the
            # array continuously busy past the ~3us ramp threshold
            wps = ps_sc.tile([128, 1024], F32, tag="sc", name="ps_warm")
            for i in range(40):
                nc.tensor.matmul(
                    wps[:, 0:128], lhsT=IDENT, rhs=IDENT,
                    start=(i == 0), stop=(i == 39),
                )

            # ---- persistent SBUF tiles (batch 0 x^T DMA'd now; batch 1
            # deferred into the first attention windows). DMA issue order is
            # chosen so the prep matmuls start as early as possible: first
            # x^T slab, weights, remaining slabs; WOUT (first read by the
            # projection ~20 windows in) goes last.
            XTs, QKVTs, VAs, OTs = [], [], [], []
            for b in range(B):
                XT = xtp.tile([128, DCX, N], MM_DT, tag="xt", name=f"xt{b}")
                if b == 0:
                    xtb = xt[b].rearrange("(dc p) n -> p dc n", p=128)
                    nc.sync.dma_start(out=XT[:, 0, :], in_=xtb[:, 0, :])
                    nc.sync.dma_start(out=WS, in_=wsel)
                    for dc in range(1, DCX):
                        nc.sync.dma_start(out=XT[:, dc, :], in_=xtb[:, dc, :])
                    nc.sync.dma_start(out=WOUT, in_=wout)
                XTs.append(XT)
                QKVTs.append(
                    [
                        qkvp.tile([128, N], MM_DT, tag=f"qkv{eb}", name=f"qkv{eb}")
                        for eb in range(3)
                    ]
                )
                # V key-blocks: [V_h0(64) | ones | V_h1(64) | ones]
                VA = vap.tile([128, KB, VAW], MM_DT, tag="va", name="va")
                nc.gpsimd.memset(VA[:, :, DP : DP + 1], 1.0)
                nc.gpsimd.memset(VA[:, :, VAW // 2 + DP : VAW // 2 + DP + 1], 1.0)
                VAs.append(VA)
                OTs.append(otp.tile([128, N], MM_DT, tag="ot", name="ot"))

            def dma_xt_slab(b2, dc):
                xtb = xt[b2].rearrange("(dc p) n -> p dc n", p=128)
                nc.sync.dma_start(out=XTs[b2][:, dc, :], in_=xtb[:, dc, :])

            # ---- PE work emitters -------------------------------------
            # All qkv-region emissions are tracked in `qkv_done` and can be
            # force-emitted on demand (ensure_qkv) by whatever reads them, so
            # emission order (= dependency order for the tile framework) is
            # structurally correct no matter how the filler queue drifts.
            qkv_done = set()
            _qtmp = {}

            def emit_qkv_use(b2, eb, nk):
                """full qkv projection block: [128, 1024] out, 16 matmuls"""
                qkv_done.add((b2, eb, nk, 0))
                qkv_done.add((b2, eb, nk, 1))
                ps = ps_sc.tile([128, 1024], F32, tag="sc", name="ps_qkv")
                for dc in range(DCX):
                    for hf in range(2):
                        nc.tensor.matmul(
                            ps[:, hf * 512 : (hf + 1) * 512],
                            lhsT=WS[:, dc, eb * 128 : (eb + 1) * 128],
                            rhs=XTs[b2][
                                :, dc,
                                nk * 1024 + hf * 512 : nk * 1024 + (hf + 1) * 512,
                            ],
                            start=(dc == 0),
                            stop=(dc == DCX - 1),
                        )
                nc.vector.tensor_copy(
                    out=QKVTs[b2][eb][:, nk * 1024 : (nk + 1) * 1024], in_=ps
                )

            def emit_qkv_half(b2, eb, nk, hf):
                """qkv projection for a 512-col slice, full contraction"""
                qkv_done.add((b2, eb, nk, hf))
                ps = ps_sc.tile([128, 1024], F32, tag="sc", name="ps_qh")
                for dc in range(DCX):
                    nc.tensor.matmul(
                        ps[:, 0:512],
                        lhsT=WS[:, dc, eb * 128 : (eb + 1) * 128],
                        rhs=XTs[b2][
                            :, dc,
                            nk * 1024 + hf * 512 : nk * 1024 + (hf + 1) * 512,
                        ],
                        start=(dc == 0),
                        stop=(dc == DCX - 1),
                    )
                nc.vector.tensor_copy(
                    out=QKVTs[b2][eb][
                        :, nk * 1024 + hf * 512 : nk * 1024 + (hf + 1) * 512
                    ],
                    in_=ps[:, 0:512],
                )

            def emit_qkv_quarter(b2, eb, nk, hf, half):
                """qkv quarter-atom: half the contraction for a 512-col slice.
                half 0 parks partials in SBUF f32; half 1 adds and stores.
                No-op if the region was already force-emitted."""
                if (b2, eb, nk, hf) in qkv_done:
                    return
                ps = ps_sc.tile([128, 1024], F32, tag="sc", name="ps_qq")
                dcs = list(range(DCH)) if half == 0 else list(range(DCH, DCX))
                for dc in dcs:
                    nc.tensor.matmul(
                        ps[:, 0:512],
                        lhsT=WS[:, dc, eb * 128 : (eb + 1) * 128],
                        rhs=XTs[b2][
                            :, dc,
                            nk * 1024 + hf * 512 : nk * 1024 + (hf + 1) * 512,
                        ],
                        start=(dc == dcs[0]),
                        stop=(dc == dcs[-1]),
                    )
                if half == 0:
                    qt = qtmpp.tile([128, 512], F32, tag="qtmp", name="qtmp")
                    nc.vector.tensor_copy(out=qt, in_=ps[:, 0:512])
                    _qtmp[b2, eb, nk, hf] = qt
                else:
                    qkv_done.add((b2, eb, nk, hf))
                    qt = _qtmp.pop((b2, eb, nk, hf))
                    nc.vector.tensor_tensor(
                        out=QKVTs[b2][eb][
                            :, nk * 1024 + hf * 512 : nk * 1024 + (hf + 1) * 512
                        ],
                        in0=ps[:, 0:512],
                        in1=qt,
                        op=mybir.AluOpType.add,
                    )

            def ensure_qkv(b2, eb, nk, hf):
                """force a qkv region into existence before a reader"""
                if (b2, eb, nk, hf) in qkv_done:
                    return
                if (b2, eb, nk, hf) in _qtmp:
                    emit_qkv_quarter(b2, eb, nk, hf, 1)
                else:
                    emit_qkv_half(b2, eb, nk, hf)

            # vtrans work is tracked in a to-do set so a PV that needs a
            # block not yet emitted can force it (emission order IS
            # dependency order for the tile framework)
            vtrans_todo = {b2: set(range(KB)) for b2 in range(B)}

            def emit_vtrans(b2, kc):
                """transpose V^T key-block kc into VA (both heads at once)"""
                if kc not in vtrans_todo[b2]:
                    return
                vtrans_todo[b2].discard(kc)
                ensure_qkv(b2, 2, kc // 8, (kc // 4) % 2)
                pst = ps_sc.tile([128, 1024], MM_DT, tag="sc", name="ps_vt")
                nc.tensor.transpose(
                    pst[:, 0:128],
                    QKVTs[b2][2][:, kc * 128 : (kc + 1) * 128],
                    IDENT,
                )
                nc.vector.tensor_copy(
                    out=VAs[b2][:, kc, 0:DP], in_=pst[:, 0:DP]
                )
                nc.vector.tensor_copy(
                    out=VAs[b2][:, kc, VAW // 2 : VAW // 2 + DP],
                    in_=pst[:, DP : 2 * DP],
                )

            def emit_proj_block(b2, nb):
                """output projection for 128 tokens"""
                py = ps_sc.tile([128, 1024], F32, tag="sc", name="py")
                for k in range(2):
                    nc.tensor.matmul(
                        py[:, k * 512 : (k + 1) * 512],
                        lhsT=OTs[b2][:, nb * 128 : (nb + 1) * 128],
                        rhs=WOUT[:, k * 512 : (k + 1) * 512],
                        start=True,
                        stop=True,
                    )
                ysb = evacp.tile([128, D], MM_DT, tag="ysb", name="ysb")
                nc.vector.tensor_copy(out=ysb, in_=py)
                nc.sync.dma_start(out=y[b2, nb * 128 : (nb + 1) * 128, :], in_=ysb)

            # ---- filler queues: primary (deadline prep work), lazy (proj).
            # Every 4th pop prefers lazy so projection blocks spread through
            # the windows instead of bunching at the batch boundary.
            primary, lazy = [], []
            _popn = [0]

            def pop_filler():
                _popn[0] += 1
                if lazy and (_popn[0] % 4 == 0 or not primary):
                    lazy.pop(0)()
                elif primary:
                    primary.pop(0)()

            # ---- prep phase: the bare minimum the first windows need —
            # K nk0 (scores kc 0-7), Q and V first 512 columns — interleaved
            # per contraction chunk so the matmuls chase the x^T slab DMAs
            # instead of serializing behind the last one. Everything else is
            # filler atoms, deadline-ordered; readers force-emit anything
            # still missing, so ordering is structurally safe.
            qkv_done.update({(0, 1, 0, 0), (0, 1, 0, 1), (0, 0, 0, 0),
                             (0, 2, 0, 0)})
            psK = ps_sc.tile([128, 1024], F32, tag="sc", name="psK")
            psQ = ps_sc.tile([128, 1024], F32, tag="sc", name="psQ")
            psV = ps_sc.tile([128, 1024], F32, tag="sc", name="psV")
            for dc in range(DCX):
                for ps2, eb, hf in (
                    (psK, 1, 0), (psK, 1, 1), (psQ, 0, 0), (psV, 2, 0),
                ):
                    nc.tensor.matmul(
                        ps2[:, hf * 512 : (hf + 1) * 512],
                        lhsT=WS[:, dc, eb * 128 : (eb + 1) * 128],
                        rhs=XTs[0][:, dc, hf * 512 : (hf + 1) * 512],
                        start=(dc == 0),
                        stop=(dc == DCX - 1),
                    )
            nc.vector.tensor_copy(out=QKVTs[0][1][:, 0:1024], in_=psK)
            nc.vector.tensor_copy(out=QKVTs[0][0][:, 0:512], in_=psQ[:, 0:512])
            nc.vector.tensor_copy(out=QKVTs[0][2][:, 0:512], in_=psV[:, 0:512])

            # b0 leftovers, deadline-ordered (~one pop per window)
            for kc in range(4):
                primary.append(lambda kc=kc: emit_vtrans(0, kc))
            for hf in range(2):      # K nk1 (scores kc=8 at window 8)
                for half in range(2):
                    primary.append(
                        lambda hf=hf, half=half: emit_qkv_quarter(0, 1, 1, hf, half)
                    )
            for half in range(2):     # V cols 512:1024
                primary.append(
                    lambda half=half: emit_qkv_quarter(0, 2, 0, 1, half)
                )
            for half in range(2):     # Q cols 512:1024
                primary.append(
                    lambda half=half: emit_qkv_quarter(0, 0, 0, 1, half)
                )
            for hf in range(2):      # V nk1 (vtrans 8+ at window ~13)
                for half in range(2):
                    primary.append(
                        lambda hf=hf, half=half: emit_qkv_quarter(0, 2, 1, hf, half)
                    )
            for kc in range(4, 16):
                primary.append(lambda kc=kc: emit_vtrans(0, kc))
            for hf in range(2):      # Q nk1 (q window 2 at window 32)
                for half in range(2):
                    primary.append(
                        lambda hf=hf, half=half: emit_qkv_quarter(0, 0, 1, hf, half)
                    )
            # b1 prep (deadline: before b1 attention)
            for eb, nk in ((1, 0), (1, 1), (0, 0), (2, 0), (2, 1)):
                for hf in range(2):
                    for half in range(2):
                        primary.append(
                            lambda eb=eb, nk=nk, hf=hf, half=half: emit_qkv_quarter(
                                1, eb, nk, hf, half
                            )
                        )
            for kc in range(16):
                primary.append(lambda kc=kc: emit_vtrans(1, kc))
            for hf in range(2):
                for half in range(2):
                    primary.append(
                        lambda hf=hf, half=half: emit_qkv_quarter(1, 0, 1, hf, half)
                    )

            # ---- attention ---------------------------------------------
            # PV matmuls and the normalize chain trail the scores/exp stream
            # through a global pending queue that carries across q-window and
            # batch boundaries, so the exp stream never pauses. Windows where
            # the queue runs deep (just after a boundary) emit two items.
            pending = []

            def emit_norm(pvs, OT, q0, last=False):
                for h in range(HC):
                    pv = pvs[h]
                    # custom-DVE reciprocal can't read PSUM and only works
                    # on partition-base-0 tiles: stage the raw denominator
                    # row to SBUF, shift to partition 0, broadcast, THEN
                    # fast-reciprocal the [64, QW] tile. The unnormalized
                    # O^T is copied out up front so the single-buffered pv
                    # psum frees for the next q-window immediately.
                    stg = normp.tile([DP + 1, QW], F32, tag="stg", name="stg")
                    nc.vector.tensor_copy(
                        out=stg[DP : DP + 1, :], in_=pv[DP : DP + 1, :]
                    )
                    if last:
                        ocp = pv[0:DP, :]
                    else:
                        ocp = normp.tile([DP, QW], F32, tag="ocp", name="ocp")
                        nc.vector.tensor_copy(out=ocp, in_=pv[0:DP, :])
                    rt = normp.tile([1, QW], F32, tag="rt", name="rt")
                    nc.sync.dma_start(out=rt, in_=stg[DP : DP + 1, :])
                    bc = normp.tile([DP, QW], F32, tag="bc", name="bc")
                    nc.gpsimd.partition_broadcast(bc, rt)
                    rc = normp.tile([DP, QW], F32, tag="rc", name="rc")
                    nc.vector.reciprocal_approx_fast(out=rc, in_=bc)
                    ots = normp.tile([DP, QW], MM_DT, tag="ots", name="ots")
                    nc.vector.tensor_mul(out=ots, in0=ocp, in1=rc)
                    nc.sync.dma_start(
                        out=OT[h * DP : (h + 1) * DP, q0 : q0 + QW], in_=ots
                    )
                # the q-window's projection blocks may only be enqueued once
                # the O^T columns they read have been emitted (program order
                # = dependency order), i.e. right here
                b2 = OTs.index(OT)
                for nb in range(q0 // 128, (q0 + QW) // 128):
                    lazy.append(lambda b2=b2, nb=nb: emit_proj_block(b2, nb))

            for b in range(B):
                QT, KT, VT = QKVTs[b]
                VA = VAs[b]
                OT = OTs[b]
                for qw in range(NQW):
                    q0 = qw * QW
                    PT = ptp.tile([128, RING, 1024], P_DT, tag="pt", name="pt")
                    pvs = [
                        ps_pv.tile([DP + 1, QW], F32, tag=f"pv{h}", name=f"pv{h}")
                        for h in range(HC)
                    ]

                    def emit_pv(kc, b=b, pvs=pvs, PT=PT, VA=VA):
                        emit_vtrans(b, kc)  # no-op unless still pending
                        for h in range(HC):
                            nc.tensor.matmul(
                                pvs[h][0 : DP + 1, :],
                                lhsT=VA[
                                    :, kc,
                                    h * (VAW // 2) : h * (VAW // 2) + DP + 1,
                                ],
                                rhs=PT[:, kc % RING, h * 512 : (h + 1) * 512],
                                start=(kc == 0),
                                stop=(kc == KB - 1),
                            )

                    for kc in range(KB):
                        ensure_qkv(b, 0, qw // 2, qw % 2)        # Q columns
                        ensure_qkv(b, 1, kc // 8, (kc // 4) % 2)  # K columns
                        S = ps_sc.tile([128, 1024], F32, tag="sc", name="s")
                        for h in range(HC):
                            nc.tensor.matmul(
                                S[:, h * 512 : (h + 1) * 512],
                                lhsT=KT[
                                    h * 64 : (h + 1) * 64,
                                    kc * 128 : (kc + 1) * 128,
                                ],
                                rhs=QT[h * 64 : (h + 1) * 64, q0 : q0 + QW],
                                start=True,
                                stop=True,
                            )
                        nc.scalar.activation(
                            out=PT[:, kc % RING, :],
                            in_=S,
                            func=mybir.ActivationFunctionType.Exp,
                            scale=1.0 / SCALE,
                        )
                        # defer b1's x^T input DMA under b0's first windows
                        if b == 0 and qw == 0 and 2 <= kc < 2 + DCX:
                            dma_xt_slab(1, kc - 2)
                        pending.append(lambda kc=kc, f=emit_pv: f(kc))
                        # steady-state: one pending item per window at lag
                        # PVLAG+1 (ring depth 8 leaves 2 windows of slack);
                        # the +1 keeps the first PV of a fresh q-window far
                        # enough behind the previous norm that the
                        # single-buffered pv psum has been read out. Pending
                        # runs BEFORE the filler pop so the norm's pv-freeing
                        # DVE copies aren't stuck behind filler evacuations,
                        # and the filler pop is skipped entirely while a
                        # boundary backlog is burning down.
                        deep = len(pending) > PVLAG + 2
                        while len(pending) > PVLAG + 1:
                            pending.pop(0)()
                        if not deep:
                            pop_filler()
                    lastq = b == B - 1 and qw == NQW - 1
                    pending.append(
                        lambda pvs=pvs, OT=OT, q0=q0, lastq=lastq: emit_norm(
                            pvs, OT, q0, last=lastq
                        )
                    )

            # drain pending PV/norm work first (the final norm gates the
            # last projection blocks), then remaining fillers; a few
            # dependency-free matmuls keep the PE clock up through the final
            # norm-chain latency
            while pending:
                pending.pop(0)()
            tps = ps_sc.tile([128, 1024], F32, tag="sc", name="ps_tw")
            for i in range(12):
                nc.tensor.matmul(
                    tps[:, 0:128], lhsT=IDENT, rhs=IDENT,
                    start=(i == 0), stop=(i == 11),
                )
            while primary or lazy:
                pop_filler()
            if debug:
                for b2 in range(B):
                    for eb in range(3):
                        nc.sync.dma_start(out=dqkv[b2, eb], in_=QKVTs[b2][eb])
                    nc.sync.dma_start(out=dva[b2], in_=VAs[b2])
                    nc.sync.dma_start(out=dot[b2], in_=OTs[b2])
    nc.finalize()
    return nc


def _get_bass(with_bias=False):
    key = f"nc{int(with_bias)}"
    if key not in _CACHE:
        _CACHE[key] = _build_bass(with_bias)
    return _CACHE[key]


def _make_in_maps(x, W_qkv, b_qkv, W_out):
    """Shard the full inputs into the 8 per-core input dicts."""
    x = np.asarray(x, dtype=np.float32)
    W_qkv = np.asarray(W_qkv, dtype=np.float32)
    b_qkv = np.asarray(b_qkv, dtype=np.float32)
    W_out = np.asarray(W_out, dtype=np.float32)

    with_bias = bool(np.any(b_qkv))
    # x^T per batch, shared by all cores (+ optional bias chunk rows)
    xtt = x.transpose(0, 2, 1)
    if with_bias:
        aug = np.zeros((B, 128, N), dtype=np.float32)
        aug[:, 0, :] = 1.0
        xtt = np.concatenate([xtt, aug], axis=1)
    xt = np.ascontiguousarray(xtt).astype(BF16)

    in_maps = []
    for c in range(NCORES):
        heads = [HC * c + i for i in range(HC)]
        # W_qkv columns: head h occupies cols [h*3*DP, (h+1)*3*DP) as [q|k|v]
        qcols = [W_qkv[:, h * 3 * DP : h * 3 * DP + DP] for h in heads]
        kcols = [W_qkv[:, h * 3 * DP + DP : h * 3 * DP + 2 * DP] for h in heads]
        vcols = [W_qkv[:, h * 3 * DP + 2 * DP : h * 3 * DP + 3 * DP] for h in heads]
        wsel = np.concatenate(qcols + kcols + vcols, axis=1)  # [D, 3*E]
        if with_bias:
            bq = [b_qkv[h * 3 * DP : h * 3 * DP + DP] for h in heads]
            bk = [b_qkv[h * 3 * DP + DP : h * 3 * DP + 2 * DP] for h in heads]
            bv = [b_qkv[h * 3 * DP + 2 * DP : h * 3 * DP + 3 * DP] for h in heads]
            brow = np.concatenate(bq + bk + bv)  # [3*E]
            baug = np.zeros((128, 3 * E), dtype=np.float32)
            baug[0, :] = brow
            wsel = np.concatenate([wsel, baug], axis=0)
        woutc = np.concatenate(
            [W_out[h * DP : (h + 1) * DP, :] for h in heads], axis=0
        )  # [E, D]
        dcx = wsel.shape[0] // 128
        wsel_r = wsel.reshape(dcx, 128, wsel.shape[1]).transpose(1, 0, 2)
        in_maps.append(
            {
                "xt": xt,
                "wsel": np.ascontiguousarray(wsel_r.reshape(128, -1)).astype(BF16),
                "wout": np.ascontiguousarray(woutc).astype(BF16),
            }
        )
    return in_maps, with_bias


def _run(in_maps, with_bias=False, trace=False):
    from concourse import bass_utils

    nc = _get_bass(with_bias)
    return bass_utils.run_bass_kernel_spmd(
        nc, in_maps, core_ids=list(range(NCORES)), trace=trace
    )


def kernel(x, W_qkv, b_qkv, W_out, b_out, _trace=False):
    in_maps, with_bias = _make_in_maps(x, W_qkv, b_qkv, W_out)
    res = _run(in_maps, with_bias=with_bias, trace=_trace)
    y = np.zeros((B, N, D), dtype=np.float32)
    for r in res.results:
        y += np.asarray(r["y"], dtype=np.float32)
    y += np.asarray(b_out, dtype=np.float32)
    _CACHE["last_result"] = res
    return y
